# revision 2
# baseline (speedup 1.0000x reference)
"""Dynamic lightweight convolution TRN2 kernel.

out[b,l,d] = (1/K) * sum_k softmax_k(x[b,l+K-1,:] @ W + bias)[k, d%16] * x[b,l+k,d]
B=8, S=2048, D=1024, K=7, H=16, L=2042. One batch element per core.

Design (~86us vs the 144us v1 baseline):
  - Host ships x^T pre-cast to fp16 [D, S]: no on-chip input transposes or
    PSUM->SBUF input copies; input DMA halved.
  - logits/softmax path as v1 (PE matmul + ACT exp + DVE recip), in fp16.
  - m[d, slot, l] = en[16*k_slot + d%16, l+6] via selector matmuls (slots
    ordered [0,2,4,6,1,3,5] so product ops can use stride-2 tap APs).
  - conv products p[slot] = m_slot * x_{+k}: fused multi-tap TensorTensor ops
    (raw APs: stride-0 chunk bcast on m, stride-2 tap dim on x), split
    DVE (2x mode) / Pool by (c, window) unit.
  - adds + output transpose fused on PE: psum[l, d] += p_slot^T via 7
    accumulating matmuls against an fp16 identity (per 128-l-window, per c).
    A few units instead run a DVE add tree + single transpose to offload PE.
  - ACT copies psum -> SBUF f32, DMA out from SBUF. Output stays f32.
"""

import numpy as np
from contextlib import ExitStack

import concourse.bacc as bacc
import concourse.tile as tile
from concourse import mybir
from concourse import bass_utils
from concourse.ap import AP as APc

K = 7
H = 16
B, S, D = 8, 2048, 1024
L = S - K + 1  # 2042
C = D // 128  # 8
NB = 4  # s-blocks
SB = S // NB  # 512
KH = K * H  # 112
SLOTS = [0, 2, 4, 6, 1, 3, 5]  # tap order in m / p_buf slots

# conv l-windows: 256-wide through l<1792, then 128 + 122 so the pipeline
# drains gradually at the tail
WL = 256  # pb tile width (max window length)
WSTART = [0, 256, 512, 768, 1024, 1280, 1536, 1792, 1920]
WEND = [256, 512, 768, 1024, 1280, 1536, 1792, 1920, 2042]
NW = len(WSTART)
WIN_OF_BLOCK = [[0], [1, 2], [3, 4], [5, 6, 7, 8]]
MBLK = [0, 506, 1018, 1530, 2042]  # mrep l-ranges (en block j covers +6)

# engine assignment per window: which c's products go to Pool, which unit
# uses the DVE add-tree path (others accumulate on PE)
POOL_CS = {w: [(2 * w) % 8, (2 * w + 1) % 8] for w in range(7)}
POOL_CS[7] = [4]
POOL_CS[8] = [6]
TREE_CS = {w: [] for w in range(NW)}
for w in (2, 3, 4, 5, 6):
    TREE_CS[w] = [(2 * w + 2) % 8]
TREE_CS[7] = [0, 2]
TREE_CS[8] = [0, 2]

F32 = mybir.dt.float32
F16 = mybir.dt.float16

# consts blob byte offsets (per partition)
_OFF_BIAS = 0  # [112, 1] f32
_OFF_IDENT = 4  # [128, 128] f16
_OFF_SELSUM = 260  # [112, 112] f16
_OFF_WT = 484  # [128, 8*112] f16
_OFF_SELK = 2276  # [112, 7*128] f16
_CONST_BYTES = 4080  # padded to /16


def _host_constants(W, b):
    buf = np.zeros((128, _CONST_BYTES), np.uint8)

    def put(off, arr):
        by = np.ascontiguousarray(arr).view(np.uint8).reshape(arr.shape[0], -1)
        buf[: arr.shape[0], off : off + by.shape[1]] = by

    put(_OFF_BIAS, np.asarray(b, np.float32).reshape(KH, 1))
    put(_OFF_IDENT, np.eye(128, dtype=np.float16))
    h = np.arange(KH) % H
    selsum = ((h[:, None] == h[None, :]) * float(K)).astype(np.float16)
    put(_OFF_SELSUM, selsum)
    selk = np.zeros((KH, K * 128), dtype=np.float16)
    for slot, k in enumerate(SLOTS):
        for p in range(128):
            selk[16 * k + p % 16, slot * 128 + p] = 1.0
    put(_OFF_SELK, selk)
    wt = np.asarray(W, np.float32).astype(np.float16)  # [D, KH]
    wt = wt.reshape(C, 128, KH).transpose(1, 0, 2).reshape(128, C * KH)
    put(_OFF_WT, np.ascontiguousarray(wt))
    return buf.view(np.float32)


def _win(t, off, dims):
    """Raw AP view of tile t: free dims = [[stride, count], ...] (elements)."""
    return APc(t.tensor, t.offset + off, [list(t.ap[0])] + [list(d) for d in dims])


def build_program():
    nc = bacc.Bacc(
        "TRN2", target_bir_lowering=False, debug=False, enable_asserts=True
    )

    xt_d = nc.dram_tensor("xt", [D, S], F16, kind="ExternalInput").ap()
    consts_d = nc.dram_tensor(
        "consts", [128, _CONST_BYTES // 4], F32, kind="ExternalInput"
    ).ap()
    out_d = nc.dram_tensor("out", [L, D], F32, kind="ExternalOutput").ap()

    with tile.TileContext(nc) as tc, ExitStack() as ctx:
        singles = ctx.enter_context(tc.tile_pool(name="singles", bufs=1))
        rinv_pool = ctx.enter_context(tc.tile_pool(name="rinvp", bufs=4))
        pb_pool = ctx.enter_context(tc.tile_pool(name="pb", bufs=3))
        tmp_pool = ctx.enter_context(tc.tile_pool(name="tmp", bufs=2))
        outn_pool = ctx.enter_context(tc.tile_pool(name="outn", bufs=3))

        p_front = ctx.enter_context(
            tc.tile_pool(name="pfront", bufs=2, space="PSUM")
        )
        p_mk = ctx.enter_context(tc.tile_pool(name="pmk", bufs=2, space="PSUM"))
        p_otp = ctx.enter_context(tc.tile_pool(name="potp", bufs=2, space="PSUM"))

        # ---- constants ----
        cblob = singles.tile([128, _CONST_BYTES // 4], F32)
        # head (bias/ident/selsum/wt) first: unblocks the PE warmup and the
        # first logits matmul; selk (only needed by mrep, ~3us later) last,
        # AFTER the first x slab so logits(0) starts as early as possible.
        nc.sync.dma_start(
            out=cblob[:, : _OFF_SELK // 4], in_=consts_d[:, : _OFF_SELK // 4]
        )
        cbytes = cblob.bitcast(mybir.dt.uint8)

        def cview(off, nbytes, dt, rows=128):
            return cbytes[:rows, off : off + nbytes].bitcast(dt)

        bias_t = cview(_OFF_BIAS, 4, F32, rows=KH)
        ident_t = cview(_OFF_IDENT, 256, F16)
        selsum_t = cview(_OFF_SELSUM, 224, F16, rows=KH)
        selk_t = cview(_OFF_SELK, 1792, F16, rows=KH).rearrange(
            "c (k p) -> c k p", k=K
        )
        wt = cview(_OFF_WT, 1792, F16).rearrange("p (c n) -> p c n", c=C)

        # PE pstate warmup: ~3us of junk matmuls so logits(0) runs at full
        # clock. Reads the consts blob (first DMA), writes a scratch bank.
        wpsum = p_mk.tile([128, SB], F32, tag="pmk")
        wident = cview(_OFF_IDENT, 256, F16)
        for _ in range(21):
            nc.tensor.matmul(
                wpsum[:, :128], wident, wident, start=True, stop=True
            )

        # warmups: GPSIMD TT library + ACT Exp table
        warm_g = singles.tile([1, 8], F16)
        nc.gpsimd.tensor_mul(warm_g, ident_t[:1, :8], ident_t[:1, :8])
        warm_e = singles.tile([1, 1], F16)
        nc.scalar.activation(
            warm_e, bias_t[:1, :1], mybir.ActivationFunctionType.Exp
        )

        # ---- persistent tensors ----
        xtb = singles.tile([128, C, S], F16)  # x^T fp16
        e_full = singles.tile([KH, S], F16)
        en = singles.tile([KH, S], F16)
        m = singles.tile([128, K, S], F16)  # m[:, slot, l]

        # ---- input DMA: slab 0, then the selk consts, then slabs 1-3 ----
        xin = xt_d.rearrange("(c p) s -> p c s", p=128)
        nc.sync.dma_start(out=xtb[:, :, :SB], in_=xin[:, :, :SB])
        nc.sync.dma_start(
            out=cblob[:, _OFF_SELK // 4 :], in_=consts_d[:, _OFF_SELK // 4 :]
        )
        for bb in range(1, NB):
            nc.sync.dma_start(
                out=xtb[:, :, SB * bb : SB * (bb + 1)],
                in_=xin[:, :, SB * bb : SB * (bb + 1)],
            )

        def front_a(s0, s1):
            """logits matmuls + exp for x columns [s0, s1)."""
            sl = slice(s0, s1)
            n = s1 - s0
            plog = p_front.tile([KH, SB], F32, tag="pfront")
            for c in range(C):
                nc.tensor.matmul(
                    plog[:, :n],
                    wt[:, c, :],
                    xtb[:, c, sl],
                    start=(c == 0),
                    stop=(c == C - 1),
                )
            nc.scalar.activation(
                e_full[:, sl],
                plog[:, :n],
                mybir.ActivationFunctionType.Exp,
                bias=bias_t,
                scale=1.0,
            )

        def front_b(s0, s1):
            """softmax denominators + normalized weights for [s0, s1)."""
            sl = slice(s0, s1)
            n = s1 - s0
            psum = p_front.tile([KH, SB], F32, tag="pfront")
            nc.tensor.matmul(
                psum[:, :n], selsum_t, e_full[:, sl], start=True, stop=True
            )
            rinvf = rinv_pool.tile([KH, SB], F32, tag="rinvf")
            # halves: the first mrep granules of the block unblock sooner
            h = n // 2
            for a, bnd in ((0, h), (h, n)):
                nc.vector.reciprocal(rinvf[:, a:bnd], psum[:, a:bnd])
                nc.vector.tensor_mul(
                    en[:, s0 + a : s0 + bnd],
                    e_full[:, s0 + a : s0 + bnd],
                    rinvf[:, a:bnd],
                )

        def mg(g):
            """mrep granule g: m[:, :, 64g : 64g+nl] via 7 selector matmuls
            into a 1-bank psum tile + one fused ACT copy (pmk double-buffers
            so consecutive granules pipeline against the copy)."""
            l0 = 64 * g
            nl = min(64, L - l0)
            pmk = p_mk.tile([128, K * 64], F32, tag="pmk")
            pv = pmk.rearrange("p (k l) -> p k l", k=K)
            for slot in range(K):
                nc.tensor.matmul(
                    pv[:, slot, :nl],
                    selk_t[:, slot, :],
                    en[:, l0 + K - 1 : l0 + K - 1 + nl],
                    start=True,
                    stop=True,
                )
            nc.scalar.copy(m[:, :, l0 : l0 + nl], pv[:, :, :nl])

        def mh(j):
            mg(2 * j)
            mg(2 * j + 1)

        pb_tiles = {}
        tmp_tiles = {}

        def products(w):
            """Emit product ops for all 8 chunks of window w."""
            lw0 = WSTART[w]
            nlw = WEND[w] - lw0
            pb = pb_pool.tile([128, C, K, WL], F16, tag="pb")
            pb_tiles[w] = pb
            pool_cs = POOL_CS[w]
            dve_cs = [c for c in range(C) if c not in pool_cs]
            # Pool units first (slowest engine; start it early)
            if len(pool_cs) == 2 and pool_cs[1] == pool_cs[0] + 1:
                c0 = pool_cs[0]
                for par, ntap, xoff in ((0, 4, 0), (4, 3, 1)):
                    nc.gpsimd.tensor_mul(
                        pb[:, c0 : c0 + 2, par : par + ntap, :nlw],
                        _win(m, par * S + lw0, [[0, 2], [S, ntap], [1, nlw]]),
                        _win(
                            xtb,
                            c0 * S + lw0 + xoff,
                            [[S, 2], [2, ntap], [1, nlw]],
                        ),
                    )
            else:
                for c in pool_cs:
                    for par, ntap, xoff in ((0, 4, 0), (4, 3, 1)):
                        nc.gpsimd.tensor_mul(
                            pb[:, c, par : par + ntap, :nlw],
                            _win(m, par * S + lw0, [[S, ntap], [1, nlw]]),
                            _win(xtb, c * S + lw0 + xoff, [[2, ntap], [1, nlw]]),
                        )
            # DVE units, fused in pairs of adjacent c where possible
            runs = []
            i = 0
            while i < len(dve_cs):
                n = 1
                while (
                    i + n < len(dve_cs) and dve_cs[i + n] == dve_cs[i] + n
                ):
                    n += 1
                runs.append((dve_cs[i], n))
                i += n
            for c0, ncc in runs:
                for par, ntap, xoff in ((0, 4, 0), (4, 3, 1)):
                    if ncc == 1:
                        out_ap = pb[:, c0, par : par + ntap, :nlw]
                        m_ap = _win(m, par * S + lw0, [[S, ntap], [1, nlw]])
                        x_ap = _win(
                            xtb, c0 * S + lw0 + xoff, [[2, ntap], [1, nlw]]
                        )
                    else:
                        out_ap = pb[:, c0 : c0 + ncc, par : par + ntap, :nlw]
                        m_ap = _win(
                            m, par * S + lw0, [[0, ncc], [S, ntap], [1, nlw]]
                        )
                        x_ap = _win(
                            xtb,
                            c0 * S + lw0 + xoff,
                            [[S, ncc], [2, ntap], [1, nlw]],
                        )
                    nc.vector.tensor_mul(out_ap, m_ap, x_ap)
            # DVE add tree for tree units -> tmp[:, 0, :]
            if not TREE_CS[w]:
                tmp_tiles[w] = None
                return
            tmp = tmp_pool.tile([128, 5, len(TREE_CS[w]), WL], F16, tag="tmp")
            tmp_tiles[w] = tmp
            for ti, c in enumerate(TREE_CS[w]):
                # X0=s0+s1, X1=s2+s3, X2=s4+s5 (one fused op)
                nc.vector.tensor_add(
                    tmp[:, 0:3, ti, :nlw],
                    _win(pb, (c * K + 0) * WL, [[2 * WL, 3], [1, nlw]]),
                    _win(pb, (c * K + 1) * WL, [[2 * WL, 3], [1, nlw]]),
                )
                # Y0 = X0+X1 -> slot3 ; Y1 = X2+s6 -> slot4 ; acc = Y0+Y1 -> slot0
                nc.vector.tensor_add(
                    tmp[:, 3, ti, :nlw], tmp[:, 0, ti, :nlw], tmp[:, 1, ti, :nlw]
                )
                nc.vector.tensor_add(
                    tmp[:, 4, ti, :nlw], tmp[:, 2, ti, :nlw], pb[:, c, 6, :nlw]
                )
                nc.vector.tensor_add(
                    tmp[:, 0, ti, :nlw], tmp[:, 3, ti, :nlw], tmp[:, 4, ti, :nlw]
                )

        def transposes(w):
            """PE transpose-accumulate + psum evacuation + DMA for window w."""
            lw0 = WSTART[w]
            nlw = WEND[w] - lw0
            pb = pb_tiles.pop(w)
            tmp = tmp_tiles.pop(w)
            tree_cs = TREE_CS[w]
            for t in range((nlw + 127) // 128):
                sub0 = 128 * t
                nl = min(128, nlw - sub0)
                if nl <= 0:
                    continue
                l0 = lw0 + sub0
                potp = p_otp.tile([128, D], F32, tag="potp")
                # PE-path chunks first; tree chunks last so a late DVE add
                # tree never head-of-line-blocks the ready transposes
                for c in [c for c in range(C) if c not in tree_cs] + tree_cs:
                    if c in tree_cs:
                        ti = tree_cs.index(c)
                        nc.tensor.matmul(
                            potp[:nl, 128 * c : 128 * (c + 1)],
                            tmp[:, 0, ti, sub0 : sub0 + nl],
                            ident_t,
                            start=True,
                            stop=True,
                        )
                    else:
                        for slot in range(K):
                            nc.tensor.matmul(
                                potp[:nl, 128 * c : 128 * (c + 1)],
                                pb[:, c, slot, sub0 : sub0 + nl],
                                ident_t,
                                start=(slot == 0),
                                stop=(slot == K - 1),
                            )
                outn = outn_pool.tile([128, D], F32, tag="outn")
                if w >= NW - 2:
                    # tail: half-copies so the DMA overlaps the second copy
                    nc.scalar.copy(outn[:nl, : D // 2], potp[:nl, : D // 2])
                    nc.sync.dma_start(
                        out=out_d[l0 : l0 + nl, : D // 2],
                        in_=outn[:nl, : D // 2],
                    )
                    nc.scalar.copy(outn[:nl, D // 2 :], potp[:nl, D // 2 :])
                    nc.sync.dma_start(
                        out=out_d[l0 : l0 + nl, D // 2 :],
                        in_=outn[:nl, D // 2 :],
                    )
                else:
                    nc.scalar.copy(outn[:nl, :], potp[:nl, :])
                    nc.sync.dma_start(
                        out=out_d[l0 : l0 + nl, :], in_=outn[:nl, :]
                    )

        # ---- emission ----
        # Engines execute strictly in emission order. The softmax front and
        # the per-128-l mrep granules run ~one window ahead of the conv
        # windows; transposes (PE) interleave so no engine's stream blocks on
        # a not-yet-ready op while ready work exists behind it.
        front_a(0, 262)
        front_b(0, 262)
        mh(0)
        mh(1)
        front_a(262, 512)
        front_b(262, 512)
        mh(2)
        front_a(512, 1024)
        products(0)
        front_b(512, 1024)
        mh(3)
        products(1)
        front_a(1024, 1536)
        transposes(0)
        mh(4)
        mh(5)
        products(2)
        front_b(1024, 1536)
        mh(6)
        transposes(1)
        mh(7)
        products(3)
        front_a(1536, 2048)
        mh(8)
        transposes(2)
        mh(9)
        products(4)
        front_b(1536, 2048)
        mh(10)
        transposes(3)
        mh(11)
        products(5)
        mh(12)
        transposes(4)
        mh(13)
        products(6)
        mh(14)
        transposes(5)
        mh(15)
        products(7)
        transposes(6)
        products(8)
        transposes(7)
        transposes(8)

    nc.compile()
    return nc


_CACHE = {}


def _get_program():
    if "nc" not in _CACHE:
        _CACHE["nc"] = build_program()
    return _CACHE["nc"]


def kernel(x, W, b):
    x = np.asarray(x, dtype=np.float32)
    assert x.shape == (B, S, D), x.shape

    nc = _get_program()
    consts = _host_constants(W, b)
    in_maps = []
    for core in range(B):
        xt = np.ascontiguousarray(x[core].T.astype(np.float16))
        in_maps.append({"xt": xt, "consts": consts})
    res = bass_utils.run_bass_kernel_spmd(nc, in_maps, core_ids=list(range(B)))
    out = np.stack([res.results[core]["out"] for core in range(B)], axis=0)
    return out


# revision 3
# speedup vs baseline: 1.0400x; 1.0400x over previous
"""Dynamic lightweight convolution TRN2 kernel.

out[b,l,d] = (1/K) * sum_k softmax_k(x[b,l+K-1,:] @ W + bias)[k, d%16] * x[b,l+k,d]
B=8, S=2048, D=1024, K=7, H=16, L=2042. One batch element per core.

Design (86us vs the 144us v1 baseline):
  - Host ships x^T pre-cast to fp16 [D, S]: no on-chip input transposes or
    PSUM->SBUF input copies; input DMA halved.
  - logits/softmax path as v1 (PE matmul + ACT exp + DVE recip), in fp16.
  - m[d, slot, l] = en[16*k_slot + d%16, l+6] via selector matmuls (slots
    ordered [0,2,4,6,1,3,5] so product ops can use stride-2 tap APs).
  - conv products p[slot] = m_slot * x_{+k}: fused multi-tap TensorTensor ops
    (raw APs: stride-0 chunk bcast on m, stride-2 tap dim on x), split
    DVE (2x mode) / Pool by (c, window) unit.
  - adds + output transpose fused on PE: psum[l, d] += p_slot^T via 7
    accumulating matmuls against an fp16 identity (per 128-l-window, per c).
    A few units instead run a DVE add tree + single transpose to offload PE.
  - ACT copies psum -> SBUF f32, DMA out from SBUF. Output stays f32.
"""

import numpy as np
from contextlib import ExitStack

import concourse.bacc as bacc
import concourse.tile as tile
from concourse import mybir
from concourse import bass_utils
from concourse.ap import AP as APc

K = 7
H = 16
B, S, D = 8, 2048, 1024
L = S - K + 1  # 2042
C = D // 128  # 8
NB = 4  # s-blocks
SB = S // NB  # 512
KH = K * H  # 112
SLOTS = [0, 2, 4, 6, 1, 3, 5]  # tap order in m / p_buf slots

# conv l-windows: 256-wide through l<1792, then 128 + 122 so the pipeline
# drains gradually at the tail
WL = 256  # pb tile width (max window length)
WSTART = [0, 256, 512, 768, 1024, 1280, 1536, 1792, 1920]
WEND = [256, 512, 768, 1024, 1280, 1536, 1792, 1920, 2042]
NW = len(WSTART)
WIN_OF_BLOCK = [[0], [1, 2], [3, 4], [5, 6, 7, 8]]
MBLK = [0, 506, 1018, 1530, 2042]  # mrep l-ranges (en block j covers +6)

# engine assignment per window: which c's products go to Pool, which unit
# uses the DVE add-tree path (others accumulate on PE)
POOL_CS = {w: [(2 * w) % 8, (2 * w + 1) % 8] for w in range(7)}
POOL_CS[7] = [4]
POOL_CS[8] = [6]
TREE_CS = {w: [] for w in range(NW)}
for w in (2, 3, 4, 5, 6):
    TREE_CS[w] = [(2 * w + 2) % 8]
TREE_CS[7] = [0, 2]
TREE_CS[8] = [0, 2]

F32 = mybir.dt.float32
F16 = mybir.dt.float16

# consts blob byte offsets (per partition)
_OFF_BIAS = 0  # [112, 1] f32
_OFF_IDENT = 4  # [128, 128] f16
_OFF_SELSUM = 260  # [112, 112] f16
_OFF_WT = 484  # [128, 8*112] f16
_OFF_SELK = 2276  # [112, 7*128] f16
_CONST_BYTES = 4080  # padded to /16


def _host_constants(W, b):
    buf = np.zeros((128, _CONST_BYTES), np.uint8)

    def put(off, arr):
        by = np.ascontiguousarray(arr).view(np.uint8).reshape(arr.shape[0], -1)
        buf[: arr.shape[0], off : off + by.shape[1]] = by

    put(_OFF_BIAS, np.asarray(b, np.float32).reshape(KH, 1))
    put(_OFF_IDENT, np.eye(128, dtype=np.float16))
    h = np.arange(KH) % H
    selsum = ((h[:, None] == h[None, :]) * float(K)).astype(np.float16)
    put(_OFF_SELSUM, selsum)
    selk = np.zeros((KH, K * 128), dtype=np.float16)
    for slot, k in enumerate(SLOTS):
        for p in range(128):
            selk[16 * k + p % 16, slot * 128 + p] = 1.0
    put(_OFF_SELK, selk)
    wt = np.asarray(W, np.float32).astype(np.float16)  # [D, KH]
    wt = wt.reshape(C, 128, KH).transpose(1, 0, 2).reshape(128, C * KH)
    put(_OFF_WT, np.ascontiguousarray(wt))
    return buf.view(np.float32)


def _win(t, off, dims):
    """Raw AP view of tile t: free dims = [[stride, count], ...] (elements)."""
    return APc(t.tensor, t.offset + off, [list(t.ap[0])] + [list(d) for d in dims])


def build_program():
    nc = bacc.Bacc(
        "TRN2", target_bir_lowering=False, debug=False, enable_asserts=True
    )

    xt_d = nc.dram_tensor("xt", [D, S], F16, kind="ExternalInput").ap()
    consts_d = nc.dram_tensor(
        "consts", [128, _CONST_BYTES // 4], F32, kind="ExternalInput"
    ).ap()
    out_d = nc.dram_tensor("out", [L, D], F32, kind="ExternalOutput").ap()

    with tile.TileContext(nc) as tc, ExitStack() as ctx:
        singles = ctx.enter_context(tc.tile_pool(name="singles", bufs=1))
        rinv_pool = ctx.enter_context(tc.tile_pool(name="rinvp", bufs=4))
        pb_pool = ctx.enter_context(tc.tile_pool(name="pb", bufs=3))
        tmp_pool = ctx.enter_context(tc.tile_pool(name="tmp", bufs=2))
        outn_pool = ctx.enter_context(tc.tile_pool(name="outn", bufs=3))

        p_front = ctx.enter_context(
            tc.tile_pool(name="pfront", bufs=2, space="PSUM")
        )
        p_mk = ctx.enter_context(tc.tile_pool(name="pmk", bufs=2, space="PSUM"))
        p_otp = ctx.enter_context(tc.tile_pool(name="potp", bufs=2, space="PSUM"))

        # ---- constants ----
        cblob = singles.tile([128, _CONST_BYTES // 4], F32)
        # head (bias/ident/selsum/wt) first: unblocks the PE warmup and the
        # first logits matmul; selk (only needed by mrep, ~3us later) last,
        # AFTER the first x slab so logits(0) starts as early as possible.
        nc.sync.dma_start(
            out=cblob[:, : _OFF_SELK // 4], in_=consts_d[:, : _OFF_SELK // 4]
        )
        cbytes = cblob.bitcast(mybir.dt.uint8)

        def cview(off, nbytes, dt, rows=128):
            return cbytes[:rows, off : off + nbytes].bitcast(dt)

        bias_t = cview(_OFF_BIAS, 4, F32, rows=KH)
        ident_t = cview(_OFF_IDENT, 256, F16)
        selsum_t = cview(_OFF_SELSUM, 224, F16, rows=KH)
        selk_t = cview(_OFF_SELK, 1792, F16, rows=KH).rearrange(
            "c (k p) -> c k p", k=K
        )
        wt = cview(_OFF_WT, 1792, F16).rearrange("p (c n) -> p c n", c=C)

        # PE pstate warmup: ~3us of junk matmuls so logits(0) runs at full
        # clock. Reads the consts blob (first DMA), writes a scratch bank.
        wpsum = p_mk.tile([128, SB], F32, tag="pmk")
        wident = cview(_OFF_IDENT, 256, F16)
        for _ in range(21):
            nc.tensor.matmul(
                wpsum[:, :128], wident, wident, start=True, stop=True
            )

        # warmups: GPSIMD TT library + ACT Exp table
        warm_g = singles.tile([1, 8], F16)
        nc.gpsimd.tensor_mul(warm_g, ident_t[:1, :8], ident_t[:1, :8])
        warm_e = singles.tile([1, 1], F16)
        nc.scalar.activation(
            warm_e, bias_t[:1, :1], mybir.ActivationFunctionType.Exp
        )

        # ---- persistent tensors ----
        xtb = singles.tile([128, C, S], F16)  # x^T fp16
        e_full = singles.tile([KH, S], F16)
        en = singles.tile([KH, S], F16)
        m = singles.tile([128, K, S], F16)  # m[:, slot, l]

        # ---- input DMA: slab 0, then the selk consts, then slabs 1-3 ----
        xin = xt_d.rearrange("(c p) s -> p c s", p=128)
        nc.sync.dma_start(out=xtb[:, :, :SB], in_=xin[:, :, :SB])
        nc.sync.dma_start(
            out=cblob[:, _OFF_SELK // 4 :], in_=consts_d[:, _OFF_SELK // 4 :]
        )
        for bb in range(1, NB):
            nc.sync.dma_start(
                out=xtb[:, :, SB * bb : SB * (bb + 1)],
                in_=xin[:, :, SB * bb : SB * (bb + 1)],
            )

        def front_a(s0, s1):
            """logits matmuls + exp for x columns [s0, s1)."""
            sl = slice(s0, s1)
            n = s1 - s0
            plog = p_front.tile([KH, SB], F32, tag="pfront")
            for c in range(C):
                nc.tensor.matmul(
                    plog[:, :n],
                    wt[:, c, :],
                    xtb[:, c, sl],
                    start=(c == 0),
                    stop=(c == C - 1),
                )
            nc.scalar.activation(
                e_full[:, sl],
                plog[:, :n],
                mybir.ActivationFunctionType.Exp,
                bias=bias_t,
                scale=1.0,
            )

        def front_b(s0, s1):
            """softmax denominators + normalized weights for [s0, s1)."""
            sl = slice(s0, s1)
            n = s1 - s0
            psum = p_front.tile([KH, SB], F32, tag="pfront")
            nc.tensor.matmul(
                psum[:, :n], selsum_t, e_full[:, sl], start=True, stop=True
            )
            rinvf = rinv_pool.tile([KH, SB], F32, tag="rinvf")
            # halves: the first mrep granules of the block unblock sooner
            h = n // 2
            for a, bnd in ((0, h), (h, n)):
                nc.vector.reciprocal(rinvf[:, a:bnd], psum[:, a:bnd])
                nc.vector.tensor_mul(
                    en[:, s0 + a : s0 + bnd],
                    e_full[:, s0 + a : s0 + bnd],
                    rinvf[:, a:bnd],
                )

        def mg(g):
            """mrep granule g: m[:, :, 64g : 64g+nl] via 7 selector matmuls
            into a 1-bank psum tile + one fused ACT copy (pmk double-buffers
            so consecutive granules pipeline against the copy)."""
            l0 = 64 * g
            nl = min(64, L - l0)
            pmk = p_mk.tile([128, K * 64], F32, tag="pmk")
            pv = pmk.rearrange("p (k l) -> p k l", k=K)
            for slot in range(K):
                nc.tensor.matmul(
                    pv[:, slot, :nl],
                    selk_t[:, slot, :],
                    en[:, l0 + K - 1 : l0 + K - 1 + nl],
                    start=True,
                    stop=True,
                )
            nc.scalar.copy(m[:, :, l0 : l0 + nl], pv[:, :, :nl])

        def mh(j):
            mg(2 * j)
            mg(2 * j + 1)

        pb_tiles = {}
        tmp_tiles = {}

        def products(w):
            """Emit product ops for all 8 chunks of window w."""
            lw0 = WSTART[w]
            nlw = WEND[w] - lw0
            pb = pb_pool.tile([128, C, K, WL], F16, tag="pb")
            pb_tiles[w] = pb
            pool_cs = POOL_CS[w]
            dve_cs = [c for c in range(C) if c not in pool_cs]
            # Pool units first (slowest engine; start it early)
            if len(pool_cs) == 2 and pool_cs[1] == pool_cs[0] + 1:
                c0 = pool_cs[0]
                for par, ntap, xoff in ((0, 4, 0), (4, 3, 1)):
                    nc.gpsimd.tensor_mul(
                        pb[:, c0 : c0 + 2, par : par + ntap, :nlw],
                        _win(m, par * S + lw0, [[0, 2], [S, ntap], [1, nlw]]),
                        _win(
                            xtb,
                            c0 * S + lw0 + xoff,
                            [[S, 2], [2, ntap], [1, nlw]],
                        ),
                    )
            else:
                for c in pool_cs:
                    for par, ntap, xoff in ((0, 4, 0), (4, 3, 1)):
                        nc.gpsimd.tensor_mul(
                            pb[:, c, par : par + ntap, :nlw],
                            _win(m, par * S + lw0, [[S, ntap], [1, nlw]]),
                            _win(xtb, c * S + lw0 + xoff, [[2, ntap], [1, nlw]]),
                        )
            # DVE units, fused in pairs of adjacent c where possible
            runs = []
            i = 0
            while i < len(dve_cs):
                n = 1
                while (
                    i + n < len(dve_cs) and dve_cs[i + n] == dve_cs[i] + n
                ):
                    n += 1
                runs.append((dve_cs[i], n))
                i += n
            for c0, ncc in runs:
                for par, ntap, xoff in ((0, 4, 0), (4, 3, 1)):
                    if ncc == 1:
                        out_ap = pb[:, c0, par : par + ntap, :nlw]
                        m_ap = _win(m, par * S + lw0, [[S, ntap], [1, nlw]])
                        x_ap = _win(
                            xtb, c0 * S + lw0 + xoff, [[2, ntap], [1, nlw]]
                        )
                    else:
                        out_ap = pb[:, c0 : c0 + ncc, par : par + ntap, :nlw]
                        m_ap = _win(
                            m, par * S + lw0, [[0, ncc], [S, ntap], [1, nlw]]
                        )
                        x_ap = _win(
                            xtb,
                            c0 * S + lw0 + xoff,
                            [[S, ncc], [2, ntap], [1, nlw]],
                        )
                    nc.vector.tensor_mul(out_ap, m_ap, x_ap)
            # DVE add tree for tree units -> tmp[:, 0, :]
            if not TREE_CS[w]:
                tmp_tiles[w] = None
                return
            tmp = tmp_pool.tile([128, 5, len(TREE_CS[w]), WL], F16, tag="tmp")
            tmp_tiles[w] = tmp
            for ti, c in enumerate(TREE_CS[w]):
                # X0=s0+s1, X1=s2+s3, X2=s4+s5 (one fused op)
                nc.vector.tensor_add(
                    tmp[:, 0:3, ti, :nlw],
                    _win(pb, (c * K + 0) * WL, [[2 * WL, 3], [1, nlw]]),
                    _win(pb, (c * K + 1) * WL, [[2 * WL, 3], [1, nlw]]),
                )
                # Y0 = X0+X1 -> slot3 ; Y1 = X2+s6 -> slot4 ; acc = Y0+Y1 -> slot0
                nc.vector.tensor_add(
                    tmp[:, 3, ti, :nlw], tmp[:, 0, ti, :nlw], tmp[:, 1, ti, :nlw]
                )
                nc.vector.tensor_add(
                    tmp[:, 4, ti, :nlw], tmp[:, 2, ti, :nlw], pb[:, c, 6, :nlw]
                )
                nc.vector.tensor_add(
                    tmp[:, 0, ti, :nlw], tmp[:, 3, ti, :nlw], tmp[:, 4, ti, :nlw]
                )

        def transposes(w):
            """PE transpose-accumulate + psum evacuation + DMA for window w."""
            lw0 = WSTART[w]
            nlw = WEND[w] - lw0
            pb = pb_tiles.pop(w)
            tmp = tmp_tiles.pop(w)
            tree_cs = TREE_CS[w]
            for t in range((nlw + 127) // 128):
                sub0 = 128 * t
                nl = min(128, nlw - sub0)
                if nl <= 0:
                    continue
                l0 = lw0 + sub0
                potp = p_otp.tile([128, D], F32, tag="potp")
                # PE-path chunks first; tree chunks last so a late DVE add
                # tree never head-of-line-blocks the ready transposes
                for c in [c for c in range(C) if c not in tree_cs] + tree_cs:
                    if c in tree_cs:
                        ti = tree_cs.index(c)
                        nc.tensor.matmul(
                            potp[:nl, 128 * c : 128 * (c + 1)],
                            tmp[:, 0, ti, sub0 : sub0 + nl],
                            ident_t,
                            start=True,
                            stop=True,
                        )
                    else:
                        for slot in range(K):
                            nc.tensor.matmul(
                                potp[:nl, 128 * c : 128 * (c + 1)],
                                pb[:, c, slot, sub0 : sub0 + nl],
                                ident_t,
                                start=(slot == 0),
                                stop=(slot == K - 1),
                            )
                outn = outn_pool.tile([128, D], F32, tag="outn")
                if w >= NW - 2:
                    # tail: half-copies so the DMA overlaps the second copy
                    nc.scalar.copy(outn[:nl, : D // 2], potp[:nl, : D // 2])
                    nc.sync.dma_start(
                        out=out_d[l0 : l0 + nl, : D // 2],
                        in_=outn[:nl, : D // 2],
                    )
                    nc.scalar.copy(outn[:nl, D // 2 :], potp[:nl, D // 2 :])
                    nc.sync.dma_start(
                        out=out_d[l0 : l0 + nl, D // 2 :],
                        in_=outn[:nl, D // 2 :],
                    )
                else:
                    nc.scalar.copy(outn[:nl, :], potp[:nl, :])
                    nc.sync.dma_start(
                        out=out_d[l0 : l0 + nl, :], in_=outn[:nl, :]
                    )

        # ---- emission ----
        # Engines execute strictly in emission order. The softmax front and
        # the per-128-l mrep granules run ~one window ahead of the conv
        # windows; transposes (PE) interleave so no engine's stream blocks on
        # a not-yet-ready op while ready work exists behind it.
        front_a(0, 262)
        front_b(0, 262)
        mh(0)
        mh(1)
        front_a(262, 512)
        front_b(262, 512)
        mh(2)
        front_a(512, 1024)
        products(0)
        front_b(512, 1024)
        mh(3)
        products(1)
        front_a(1024, 1536)
        transposes(0)
        mh(4)
        mh(5)
        products(2)
        front_b(1024, 1536)
        mh(6)
        transposes(1)
        mh(7)
        products(3)
        front_a(1536, 2048)
        mh(8)
        transposes(2)
        mh(9)
        products(4)
        front_b(1536, 2048)
        mh(10)
        transposes(3)
        mh(11)
        products(5)
        mh(12)
        transposes(4)
        mh(13)
        products(6)
        mh(14)
        transposes(5)
        mh(15)
        products(7)
        transposes(6)
        products(8)
        transposes(7)
        transposes(8)

    nc.compile()
    return nc


_CACHE = {}


def _get_program():
    if "nc" not in _CACHE:
        _CACHE["nc"] = build_program()
    return _CACHE["nc"]


def kernel(x, W, b):
    x = np.asarray(x, dtype=np.float32)
    assert x.shape == (B, S, D), x.shape

    nc = _get_program()
    consts = _host_constants(W, b)
    in_maps = []
    for core in range(B):
        xt = np.ascontiguousarray(x[core].T.astype(np.float16))
        in_maps.append({"xt": xt, "consts": consts})
    res = bass_utils.run_bass_kernel_spmd(nc, in_maps, core_ids=list(range(B)))
    out = np.stack([res.results[core]["out"] for core in range(B)], axis=0)
    return out
